# revision 5
# baseline (speedup 1.0000x reference)
import os
import sys
from contextlib import ExitStack

import numpy as np

sys.path.insert(0, "/opt/trn_rl_repo")

import concourse.bacc as bacc
import concourse.bass as bass
import concourse.tile as tile
from concourse import mybir
from concourse.bass_utils import run_bass_kernel_spmd
from concourse.masks import make_identity

# NoisyTopkRouter: B=4, S=4096, D=2048, 64 experts, top-8.
# Sharding: data-parallel over batch*seq -> 8 cores x 2048 tokens.
# Layout: host pre-transposes h to [d, tok] chunks so the contraction dim
# lands on SBUF partitions; fused W = concat(W_gate, W_noise) -> one matmul
# pass produces both logit sets ([exp|noise] on the 128 stationary rows).
B, S, D = 4, 4096, 2048
E = 64            # experts
EW = 128          # fused gate+noise stationary rows
TOPK = 8
NCORES = 8
TPC = (B * S) // NCORES   # tokens per core = 2048
G = 4             # token groups per core (512 each)
GT = 512          # tokens per group (= max moving free dim = 1 PSUM bank fp32)
CH = D // 128     # 16 contraction chunks of 128
SUB = GT // 128   # 4 subtiles of 128 tokens

F32 = mybir.dt.float32
AF = mybir.ActivationFunctionType

USE_F32R = os.environ.get("NTR_F32R", "0") == "1"

_NC_CACHE = {}


def _build_nc(use_f32r: bool) -> bass.Bass:
    # Bacc's finalize() runs the TRN2 sync-wait splitting passes (HW allows
    # at most 1 wait per instruction); plain Bass fails walrus codegen
    nc = bacc.Bacc()
    ht = nc.declare_dram_parameter("ht", (G, 128, CH, GT), F32, isOutput=False)
    wt = nc.declare_dram_parameter("wt", (128, CH, EW), F32, isOutput=False)
    nz = nc.declare_dram_parameter("noise", (G, 128, SUB, E), F32, isOutput=False)
    go = nc.declare_dram_parameter("gates", (G, 128, SUB, E), F32, isOutput=True)
    vo = nc.declare_dram_parameter("vals", (G, 128, SUB, TOPK), F32, isOutput=True)
    io_ = nc.declare_dram_parameter("inds", (G, 128, SUB, TOPK), mybir.dt.uint32,
                                    isOutput=True)

    with tile.TileContext(nc) as tc, ExitStack() as ctx:
        consts = ctx.enter_context(tc.tile_pool(name="consts", bufs=1))
        hpool = ctx.enter_context(tc.tile_pool(name="hpool", bufs=2))
        npool = ctx.enter_context(tc.tile_pool(name="npool", bufs=2))
        ltpool = ctx.enter_context(tc.tile_pool(name="ltpool", bufs=2))
        mmps = ctx.enter_context(tc.tile_pool(name="mmps", bufs=2, space="PSUM"))
        tpps = ctx.enter_context(tc.tile_pool(name="tpps", bufs=4, space="PSUM"))
        work = ctx.enter_context(tc.tile_pool(name="work", bufs=18))
        gop = ctx.enter_context(tc.tile_pool(name="gop", bufs=2))
        vop = ctx.enter_context(tc.tile_pool(name="vop", bufs=2))
        iop = ctx.enter_context(tc.tile_pool(name="iop", bufs=2))

        wt_sb = consts.tile([128, CH, EW], F32)
        nc.sync.dma_start(out=wt_sb[:], in_=wt[:])
        ident = consts.tile([128, 128], F32)
        make_identity(nc, ident)

        for g in range(G):
            ht_g = hpool.tile([128, CH, GT], F32)
            half = CH // 2
            nc.sync.dma_start(out=ht_g[:, 0:half, :], in_=ht[g, :, 0:half, :])
            nc.scalar.dma_start(out=ht_g[:, half:CH, :], in_=ht[g, :, half:CH, :])
            nz_g = npool.tile([128, SUB, E], F32)
            nc.gpsimd.dma_start(out=nz_g[:], in_=nz[g])

            # logits^T [exp|noise, tok] accumulated over d chunks
            mm = mmps.tile([128, GT], F32)
            for c in range(CH):
                lhs = wt_sb[:, c, :]
                rhs = ht_g[:, c, :]
                if use_f32r:
                    lhs = lhs.bitcast(mybir.dt.float32r)
                    rhs = rhs.bitcast(mybir.dt.float32r)
                nc.tensor.matmul(mm[:], lhsT=lhs, rhs=rhs,
                                 start=(c == 0), stop=(c == CH - 1))

            lt = ltpool.tile([128, GT], F32)
            nc.scalar.copy(lt[:], mm[:])

            gates_g = gop.tile([128, SUB, E], F32)
            vals_g = vop.tile([128, SUB, TOPK], F32)
            inds_g = iop.tile([128, SUB, TOPK], mybir.dt.uint32)

            for s in range(SUB):
                tp = tpps.tile([128, 128], F32)
                nc.tensor.transpose(tp[:], lt[:, s * 128:(s + 1) * 128], ident[:])
                gl = tp[:, 0:E]      # gate logits [tok, exp] (PSUM)
                nl = tp[:, E:EW]     # noise logits

                # softplus(x) = relu(x) + ln(1 + exp(-|x|)); all funcs live in
                # the natural_log_exp_and_others ACT table -> no table swaps
                ab = work.tile([128, E], F32)
                nc.scalar.activation(ab[:], nl, AF.Abs)
                en = work.tile([128, E], F32)
                nc.scalar.activation(en[:], ab[:], AF.Exp, scale=-1.0)
                lnt = work.tile([128, E], F32)
                nc.scalar.activation(lnt[:], en[:], AF.Ln, bias=1.0)
                rl = work.tile([128, E], F32)
                nc.scalar.activation(rl[:], nl, AF.Relu)
                scale = work.tile([128, E], F32)
                nc.vector.tensor_add(scale[:], rl[:], lnt[:])

                nsc = work.tile([128, E], F32)
                nc.vector.tensor_mul(nsc[:], nz_g[:, s, :], scale[:])
                logits = work.tile([128, E], F32)
                nc.vector.tensor_add(logits[:], gl, nsc[:])

                stats = work.tile([128, 4], F32)
                nc.vector.reduce_max(stats[:, 0:1], logits[:],
                                     axis=mybir.AxisListType.X, negate=True)
                exps = work.tile([128, E], F32)
                nc.scalar.activation(exps[:], logits[:], AF.Exp,
                                     bias=stats[:, 0:1], accum_out=stats[:, 1:2])
                nc.vector.reciprocal(stats[:, 2:3], stats[:, 1:2])
                nc.vector.tensor_scalar_mul(gates_g[:, s, :], exps[:], stats[:, 2:3])
                nc.vector.max(vals_g[:, s, :], gates_g[:, s, :])
                nc.vector.max_index(inds_g[:, s, :], vals_g[:, s, :], gates_g[:, s, :])

            nc.gpsimd.dma_start(out=go[g], in_=gates_g[:])
            nc.gpsimd.dma_start(out=vo[g], in_=vals_g[:])
            nc.gpsimd.dma_start(out=io_[g], in_=inds_g[:])

    nc.finalize()
    return nc


def _prep_inputs(hidden_states, W_gate, W_noise, noise):
    h = np.ascontiguousarray(hidden_states.reshape(B * S, D))
    nzf = np.ascontiguousarray(noise.reshape(B * S, E))
    wf = np.concatenate([W_gate, W_noise], axis=0)          # [128, D]
    # [d, e] -> chunks: wt[p, c, e] = wf[e, c*128+p]
    wt_host = np.ascontiguousarray(wf.T.reshape(CH, 128, EW).transpose(1, 0, 2))
    in_maps = []
    for i in range(NCORES):
        hc = h[i * TPC:(i + 1) * TPC]                        # [2048 tok, D]
        # ht[g, p, c, t] = hc[g*512+t, c*128+p]
        ht = np.ascontiguousarray(
            hc.reshape(G, GT, CH, 128).transpose(0, 3, 2, 1))
        nzc = np.ascontiguousarray(
            nzf[i * TPC:(i + 1) * TPC].reshape(G, SUB, 128, E).transpose(0, 2, 1, 3))
        in_maps.append({"ht": ht, "wt": wt_host, "noise": nzc})
    return in_maps


def run(hidden_states, W_gate, W_noise, noise, trace=False):
    key = ("f32r" if USE_F32R else "f32")
    if key not in _NC_CACHE:
        _NC_CACHE[key] = _build_nc(USE_F32R)
    nc = _NC_CACHE[key]
    in_maps = _prep_inputs(hidden_states, W_gate, W_noise, noise)
    res = run_bass_kernel_spmd(nc, in_maps, list(range(NCORES)), trace=trace)

    vals = np.empty((B * S, TOPK), np.float32)
    inds = np.empty((B * S, TOPK), np.int32)
    gates = np.empty((B * S, E), np.float32)
    for i, r in enumerate(res.results):
        sl = slice(i * TPC, (i + 1) * TPC)
        # [G, 128, SUB, X] -> [G, SUB, 128, X] -> [TPC, X]
        gates[sl] = r["gates"].transpose(0, 2, 1, 3).reshape(TPC, E)
        vals[sl] = r["vals"].transpose(0, 2, 1, 3).reshape(TPC, TOPK)
        inds[sl] = r["inds"].view(np.int32).transpose(0, 2, 1, 3).reshape(TPC, TOPK)
    out = (vals.reshape(B, S, TOPK), inds.reshape(B, S, TOPK),
           gates.reshape(B, S, E))
    return (out, res) if trace else out


def kernel(hidden_states, W_gate, W_noise, noise):
    return run(hidden_states, W_gate, W_noise, noise, trace=False)
